# revision 3
# baseline (speedup 1.0000x reference)
"""Bahdanau additive attention on 8 Trainium2 NeuronCores.

Problem shapes (hardcoded):
  encoder_out_seq [B=4, Te=512, He=128] f32
  decoder_out_seq [B=4, Td=256, Hd=256] f32
  W_a [128, 128], U_a [256, 128], V_a [128, 1] f32
Returns (c [B, Td, He], e [B, Td, Te]) matching the jax reference.

Sharding: 8 cores = batch (4) x decoder-half (2). Each core computes a
[Td_core=128] slice of decoder steps for one batch element; weights and the
encoder sequence are replicated per core. No collectives.

Per-core algorithm (all on one NeuronCore, Tile-scheduled):
  WsT[f,s]  = W_a^T @ enc^T           (PE, fp32)
  UhT[f,t]  = U_a^T @ dec^T           (PE, fp32)
  Z[f,(t,s)] = WsT[f,s] + UhT[f,t]    (DVE tensor_scalar, per-partition bias)
  feat      = tanh(Z) -> bf16         (ACT, big tiles)
  eT[s,t]   = feat_t^T @ v            (PE, N=1 column matmuls into PSUM)
  softmax over s without max-subtraction (scores are O(1) by construction):
    exp on ACT, denominator via DVE row-reduce after PE transpose,
    normalize c and e by 1/denom with per-partition tensor_scalar.
  c[t,:]    = (exp_eT)^T @ enc / denom (PE fp32 + DVE scale)
"""

import numpy as np

import concourse.bass as bass
import concourse.bacc as bacc
import concourse.tile as tile
from concourse import mybir
from concourse.bass_utils import run_bass_kernel_spmd
from concourse.masks import make_identity

B, TE, TD, HE, HD = 4, 512, 256, 128, 256
N_CORES = 8
TD_CORE = TD // 2  # 128 decoder steps per core
S_TILES = TE // 128  # 4
TBLK = 16  # decoder steps per tanh mega-tile
NBLK = TD_CORE // TBLK

F32 = mybir.dt.float32
BF16 = mybir.dt.bfloat16
AF = mybir.ActivationFunctionType


def _build():
    from contextlib import ExitStack

    nc = bacc.Bacc("TRN2", target_bir_lowering=False, debug=False)

    enc_d = nc.dram_tensor("enc", [TE, HE], F32, kind="ExternalInput").ap()
    dec_d = nc.dram_tensor("dec", [TD_CORE, HD], F32, kind="ExternalInput").ap()
    wa_d = nc.dram_tensor("wa", [HE, HE], F32, kind="ExternalInput").ap()
    ua_d = nc.dram_tensor("ua", [HD, HE], F32, kind="ExternalInput").ap()
    va_d = nc.dram_tensor("va", [HE, 1], F32, kind="ExternalInput").ap()
    c_d = nc.dram_tensor("c", [TD_CORE, HE], F32, kind="ExternalOutput").ap()
    e_d = nc.dram_tensor("e", [TD_CORE, TE], F32, kind="ExternalOutput").ap()

    with tile.TileContext(nc) as tc:
        with ExitStack() as ctx:
            consts = ctx.enter_context(tc.tile_pool(name="consts", bufs=1))
            zpool = ctx.enter_context(tc.tile_pool(name="zpool", bufs=2))
            fpool = ctx.enter_context(tc.tile_pool(name="fpool", bufs=2))
            opool = ctx.enter_context(tc.tile_pool(name="opool", bufs=1))
            ps_e = ctx.enter_context(tc.tile_pool(name="ps_e", bufs=1, space="PSUM"))
            ps_out = ctx.enter_context(
                tc.tile_pool(name="ps_out", bufs=1, space="PSUM")
            )

            # ---- load inputs ----
            ident = consts.tile([128, 128], F32, tag="ident")
            make_identity(nc, ident[:])

            enc_sb = []  # 4 tiles [128 s, 128 e]
            for i in range(S_TILES):
                t_ = consts.tile([128, HE], F32, tag=f"enc{i}")
                nc.sync.dma_start(out=t_[:], in_=enc_d[128 * i : 128 * (i + 1), :])
                enc_sb.append(t_)

            dec_sb = consts.tile([TD_CORE, HD], F32, tag="dec")
            nc.sync.dma_start(out=dec_sb[:], in_=dec_d[:])

            wa_sb = consts.tile([HE, HE], F32, tag="wa")
            nc.sync.dma_start(out=wa_sb[:], in_=wa_d[:])

            ua_sb = []  # 2 tiles [128 d, 128 f]
            for i in range(HD // 128):
                t_ = consts.tile([128, HE], F32, tag=f"ua{i}")
                nc.sync.dma_start(out=t_[:], in_=ua_d[128 * i : 128 * (i + 1), :])
                ua_sb.append(t_)

            va_sb = consts.tile([HE, 1], F32, tag="va")
            nc.sync.dma_start(out=va_sb[:], in_=va_d[:])

            v_bf = consts.tile([HE, 1], BF16, tag="v_bf")
            nc.vector.tensor_copy(out=v_bf[:], in_=va_sb[:])

            wst_sb = consts.tile([HE, TE], F32, tag="wst_sb")  # [f, s]
            uht_sb = consts.tile([HE, TD_CORE], F32, tag="uht_sb")  # [f, t]
            with tc.tile_pool(name="ps_pre", bufs=1, space="PSUM") as ps_pre:
                # ---- encT via PE transposes, then WsT = W_a^T @ encT ----
                encT_sb = consts.tile([HE, TE], F32, tag="encT")  # [e, s]
                for i in range(S_TILES):
                    pt = ps_pre.tile([128, 128], F32, tag=f"tr{i % 2}")
                    nc.tensor.transpose(
                        out=pt[:], in_=enc_sb[i][:], identity=ident[:]
                    )
                    nc.vector.tensor_copy(
                        out=encT_sb[:, 128 * i : 128 * (i + 1)], in_=pt[:]
                    )

                wst_ps = ps_pre.tile([HE, TE], F32, tag="wst")
                nc.tensor.matmul(
                    out=wst_ps[:],
                    lhsT=wa_sb[:],
                    rhs=encT_sb[:],
                    start=True,
                    stop=True,
                )
                nc.vector.tensor_copy(out=wst_sb[:], in_=wst_ps[:])

                # ---- decT via PE transposes, then UhT = U_a^T @ decT ----
                uht_ps = ps_pre.tile([HE, TD_CORE], F32, tag="uht")
                for i in range(HD // 128):
                    pt = ps_pre.tile([128, 128], F32, tag=f"tr{i % 2}")
                    nc.tensor.transpose(
                        out=pt[:],
                        in_=dec_sb[:, 128 * i : 128 * (i + 1)],
                        identity=ident[:],
                    )
                    dT = consts.tile([128, TD_CORE], F32, tag=f"decT{i}")
                    nc.vector.tensor_copy(out=dT[:], in_=pt[:])
                    nc.tensor.matmul(
                        out=uht_ps[:],
                        lhsT=ua_sb[i][:],
                        rhs=dT[:],
                        start=(i == 0),
                        stop=(i == HD // 128 - 1),
                    )
                nc.vector.tensor_copy(out=uht_sb[:], in_=uht_ps[:])

            # ---- main loop: Z = WsT + UhT[:,t]; feat = tanh(Z); eT columns ----
            # e_ps free layout: index i*128 + t holds eT[s in tile i, t]
            e_ps = ps_e.tile([128, TE], F32, tag="e_ps")
            for blk in range(NBLK):
                z = zpool.tile([128, TBLK * TE], F32, tag="z")
                for j in range(TBLK):
                    t = blk * TBLK + j
                    nc.vector.tensor_scalar_add(
                        out=z[:, TE * j : TE * (j + 1)],
                        in0=wst_sb[:],
                        scalar1=uht_sb[:, t : t + 1],
                    )
                feat = fpool.tile([128, TBLK * TE], BF16, tag="feat")
                nc.scalar.activation(out=feat[:], in_=z[:], func=AF.Tanh)
                for j in range(TBLK):
                    t = blk * TBLK + j
                    for i in range(S_TILES):
                        nc.tensor.matmul(
                            out=e_ps[:, 128 * i + t : 128 * i + t + 1],
                            lhsT=feat[:, TE * j + 128 * i : TE * j + 128 * (i + 1)],
                            rhs=v_bf[:],
                            start=True,
                            stop=True,
                        )

            # ---- softmax (no max subtraction; scores are O(1)) ----
            expT_sb = opool.tile([128, TE], F32, tag="expT")  # [(i,t) layout]
            nc.scalar.activation(out=expT_sb[:], in_=e_ps[:], func=AF.Exp)

            # transpose to [t, s] layout
            exp_ts_ps = ps_out.tile([128, TE], F32, tag="exp_ts")
            for i in range(S_TILES):
                nc.tensor.transpose(
                    out=exp_ts_ps[:, 128 * i : 128 * (i + 1)],
                    in_=expT_sb[:, 128 * i : 128 * (i + 1)],
                    identity=ident[:],
                )

            denom = opool.tile([128, 1], F32, tag="denom")
            nc.vector.reduce_sum(
                out=denom[:], in_=exp_ts_ps[:], axis=mybir.AxisListType.X
            )
            rcol = opool.tile([128, 1], F32, tag="rcol")
            nc.vector.reciprocal(out=rcol[:], in_=denom[:])

            # e output: exp[t,s] / denom[t]
            e_sm = opool.tile([128, TE], F32, tag="e_sm")
            nc.vector.tensor_scalar_mul(
                out=e_sm[:], in0=exp_ts_ps[:], scalar1=rcol[:]
            )
            nc.sync.dma_start(out=e_d[:], in_=e_sm[:])

            # c output: (sum_s exp_eT[s,t] * enc[s,:]) / denom[t]
            c_ps = ps_out.tile([TD_CORE, HE], F32, tag="c_ps")
            for i in range(S_TILES):
                nc.tensor.matmul(
                    out=c_ps[:],
                    lhsT=expT_sb[:, 128 * i : 128 * (i + 1)],
                    rhs=enc_sb[i][:],
                    start=(i == 0),
                    stop=(i == S_TILES - 1),
                )
            c_sb = opool.tile([TD_CORE, HE], F32, tag="c_sb")
            nc.vector.tensor_scalar_mul(out=c_sb[:], in0=c_ps[:], scalar1=rcol[:])
            nc.sync.dma_start(out=c_d[:], in_=c_sb[:])

    nc.compile()
    return nc


_NC = None


def _get_nc():
    global _NC
    if _NC is None:
        _NC = _build()
    return _NC


def make_in_maps(encoder_out_seq, decoder_out_seq, W_a, U_a, V_a):
    enc = np.ascontiguousarray(np.asarray(encoder_out_seq, dtype=np.float32))
    dec = np.ascontiguousarray(np.asarray(decoder_out_seq, dtype=np.float32))
    wa = np.ascontiguousarray(np.asarray(W_a, dtype=np.float32))
    ua = np.ascontiguousarray(np.asarray(U_a, dtype=np.float32))
    va = np.ascontiguousarray(np.asarray(V_a, dtype=np.float32))
    in_maps = []
    for core in range(N_CORES):
        b, h = core // 2, core % 2
        in_maps.append(
            {
                "enc": enc[b],
                "dec": np.ascontiguousarray(
                    dec[b, h * TD_CORE : (h + 1) * TD_CORE, :]
                ),
                "wa": wa,
                "ua": ua,
                "va": va,
            }
        )
    return in_maps


def assemble(results):
    c = np.zeros((B, TD, HE), dtype=np.float32)
    e = np.zeros((B, TD, TE), dtype=np.float32)
    for core in range(N_CORES):
        b, h = core // 2, core % 2
        c[b, h * TD_CORE : (h + 1) * TD_CORE, :] = results[core]["c"]
        e[b, h * TD_CORE : (h + 1) * TD_CORE, :] = results[core]["e"]
    return c, e


def kernel(encoder_out_seq, decoder_out_seq, W_a, U_a, V_a):
    nc = _get_nc()
    in_maps = make_in_maps(encoder_out_seq, decoder_out_seq, W_a, U_a, V_a)
    res = run_bass_kernel_spmd(nc, in_maps, list(range(N_CORES)))
    return assemble(res.results)
